# revision 8
# baseline (speedup 1.0000x reference)
"""Bass kernel builder for nn_Actor (pointer-network greedy decoder).

Per-core shard: B=256 rows, N=200 nodes, F=2, H=128, T=199 greedy steps.

Layouts (v3, fp32 sweep + f32r matmuls):
  - col axis n-interleaved per chunk: chunk ci covers 8 b-rows
    (B0=8ci..8ci+7); col = ci*1600 + n*8 + b_local. This makes the
    q-broadcast operand of the attention add packed (last dim = b, stride
    1, bf16) so the DVE runs the add in 2x mode.
  - fp (W1 @ encoded features) stored bf16, ALL 32 chunks SBUF-resident
    (100KB/partition) -> no HBM streaming in the decode loop.
  - zc tiles hold a PAIR of chunks; tanh runs once per pair ([H, 3200]).
  - v-dot: 4 column-tiled bf16 matmuls per chunk (b-pair groups, strided
    rhs view), PSUM out fp32; att assembled (b%128 partitions, b2, n) by
    SBUF->SBUF DMA per chunk.
  - GRU state stays fp32; q is cast to bf16 for the sweep.
"""
import numpy as np
import concourse.bass as bass
import concourse.mybir as mybir
from concourse import bacc
from concourse.tile import TileContext
from concourse.bass import ds

F32 = mybir.dt.float32
F32R = mybir.dt.float32r
BF16 = mybir.dt.bfloat16
U32 = mybir.dt.uint32
AF = mybir.ActivationFunctionType
OP = mybir.AluOpType

B = 256
N = 200
F = 2
H = 128
COLS = B * N
NEG = -1.0e30


def build(T_steps=N - 1, CHUNK=1600, NRES=15, zring=2, fpring=3, dve_k=15):
    assert CHUNK == 1600
    chunks = [(st, CHUNK) for st in range(0, COLS, CHUNK)]
    NCH = len(chunks)  # 32
    GS = CHUNK // 4    # 400 cols per matmul group = 2 b-rows
    # Chunks right after a step/half boundary (0,1,16,17) go to gpsimd: their
    # broadcast-add must not queue behind the argmax tail chain on DVE, or the
    # in-order PE queue stalls per step waiting for chunk 0's zc.
    dve_set = set(np.linspace(2, NCH - 1, dve_k).round().astype(int).tolist())
    dve_set -= {16, 17}

    nc = bacc.Bacc("TRN2", target_bir_lowering=False)

    raw_ext = nc.declare_dram_parameter("raw_features", [B, N, F], F32, isOutput=False)
    fi_ext = nc.declare_dram_parameter("first_input", [1, 1, F], F32, isOutput=False)
    Wse_ext = nc.declare_dram_parameter("W_se", [H, F], F32, isOutput=False)
    bse_ext = nc.declare_dram_parameter("b_se", [H], F32, isOutput=False)
    Wde_ext = nc.declare_dram_parameter("W_de", [H, F], F32, isOutput=False)
    bde_ext = nc.declare_dram_parameter("b_de", [H], F32, isOutput=False)
    Wih_ext = nc.declare_dram_parameter("W_ih", [3 * H, H], F32, isOutput=False)
    Whh_ext = nc.declare_dram_parameter("W_hh", [3 * H, H], F32, isOutput=False)
    bih_ext = nc.declare_dram_parameter("b_ih", [3 * H], F32, isOutput=False)
    bhh_ext = nc.declare_dram_parameter("b_hh", [3 * H], F32, isOutput=False)
    W1_ext = nc.declare_dram_parameter("W1", [H, H], F32, isOutput=False)
    W2_ext = nc.declare_dram_parameter("W2", [H, H], F32, isOutput=False)
    v_ext = nc.declare_dram_parameter("v", [H], F32, isOutput=False)
    out_ext = nc.declare_dram_parameter("out", [2, B, T_steps], F32, isOutput=True)

    with TileContext(nc) as tc:
        with (
            tc.tile_pool(name="const", bufs=1) as cpool,
            tc.tile_pool(name="zr", bufs=zring) as zpool,
            tc.tile_pool(name="fr", bufs=fpring) as fpool,
            tc.tile_pool(name="ar", bufs=3) as apool,
            tc.tile_pool(name="small", bufs=2) as spool,
            tc.tile_pool(name="pbig", bufs=3, space="PSUM") as ppool,
            tc.tile_pool(name="pone", bufs=1, space="PSUM") as ppool1,
            tc.tile_pool(name="dram", bufs=1, space="DRAM") as dpool,
        ):
            # ------------- persistent tiles -------------
            fp_res = cpool.tile([H, NRES * CHUNK], F32, name="fp_res")
            att_sb = cpool.tile([128, 2, N], F32, name="att_sb")
            m_sb = cpool.tile([128, 2, N], F32, name="m_sb")
            maskadd = cpool.tile([128, 2, N], F32, name="maskadd")
            iota_u = cpool.tile([128, N], U32, name="iota_u")
            iota_f = cpool.tile([128, N], F32, name="iota_f")
            rawT = cpool.tile([128, 2, F, N], F32, name="rawT")
            h_t = [cpool.tile([H, 128], F32, name=f"h_t{i}") for i in range(2)]
            dec_t = [cpool.tile([F, 128], F32, name=f"dec_t{i}") for i in range(2)]
            q_t = [cpool.tile([H, 128], F32, name=f"q_t{i}") for i in range(2)]
            idxbuf = [cpool.tile([128, T_steps], F32, name=f"idxbuf{b2}") for b2 in range(2)]
            sexpbuf = [cpool.tile([128, T_steps], F32, name=f"sexpbuf{b2}") for b2 in range(2)]
            idxf = [cpool.tile([128, 1], F32, name=f"idxf{b2}") for b2 in range(2)]
            oh_t = [cpool.tile([128, N], F32, name=f"oh{b2}") for b2 in range(2)]
            gprod = [cpool.tile([128, F, N], F32, name=f"gprod{b2}") for b2 in range(2)]
            dec_b = [cpool.tile([128, F], F32, name=f"dec_b{b2}") for b2 in range(2)]
            mx8 = [cpool.tile([128, 8], F32, name=f"mx8{b2}") for b2 in range(2)]
            ix8 = [cpool.tile([128, 8], U32, name=f"ix8{b2}") for b2 in range(2)]
            negmax = [cpool.tile([128, 1], F32, name=f"negmax{b2}") for b2 in range(2)]

            ident = cpool.tile([128, 128], F32, name="ident")
            WseT = cpool.tile([F, H], F32, name="WseT")
            WdeT = cpool.tile([F, H], F32, name="WdeT")
            WihT = cpool.tile([H, 3 * H], F32, name="WihT")
            WhhT = cpool.tile([H, 3 * H], F32, name="WhhT")
            W1T = cpool.tile([H, H], F32, name="W1T")
            W2T = cpool.tile([H, H], F32, name="W2T")
            v_sb = cpool.tile([H, 1], F32, name="v_sb")
            v32 = cpool.tile([H, 32], F32, name="v32")
            bse_c = cpool.tile([H, 1], F32, name="bse_c")
            bde_c = cpool.tile([H, 1], F32, name="bde_c")
            b_r = cpool.tile([H, 1], F32, name="b_r")
            b_z = cpool.tile([H, 1], F32, name="b_z")
            b_in = cpool.tile([H, 1], F32, name="b_in")
            b_hn = cpool.tile([H, 1], F32, name="b_hn")
            fi_sb = cpool.tile([F, 1], F32, name="fi_sb")

            emb_sb = [cpool.tile([H, 128], F32, name=f"emb_sb{i}") for i in range(2)]
            r_col = [cpool.tile([H, 128], F32, name=f"r_col{i}") for i in range(2)]
            z_col = [cpool.tile([H, 128], F32, name=f"z_col{i}") for i in range(2)]
            hn_sb = [cpool.tile([H, 128], F32, name=f"hn_sb{i}") for i in range(2)]
            rhn = [cpool.tile([H, 128], F32, name=f"rhn{i}") for i in range(2)]
            npre = [cpool.tile([H, 128], F32, name=f"npre{i}") for i in range(2)]
            n_col = [cpool.tile([H, 128], F32, name=f"n_col{i}") for i in range(2)]
            omz = [cpool.tile([H, 128], F32, name=f"omz{i}") for i in range(2)]
            a1 = [cpool.tile([H, 128], F32, name=f"a1_{i}") for i in range(2)]
            a2 = [cpool.tile([H, 128], F32, name=f"a2_{i}") for i in range(2)]

            fp_dram = dpool.tile([H, COLS], F32, name="fp_dram")
            rawxy_dram = dpool.tile([F, COLS], F32, name="rawxy_dram")

            # ================= PRECOMPUTE =================
            nc.gpsimd.iota(iota_u[:], pattern=[[1, N]], base=0, channel_multiplier=0)
            nc.vector.tensor_copy(iota_f[:], iota_u[:])
            idiag = cpool.tile([128, 1], U32, name="idiag")
            nc.gpsimd.iota(idiag[:], pattern=[[0, 1]], base=0, channel_multiplier=1)
            idiagf = cpool.tile([128, 1], F32, name="idiagf")
            nc.vector.tensor_copy(idiagf[:], idiag[:])
            iota128u = cpool.tile([128, 128], U32, name="iota128u")
            nc.gpsimd.iota(iota128u[:], pattern=[[1, 128]], base=0, channel_multiplier=0)
            iota128f = cpool.tile([128, 128], F32, name="iota128f")
            nc.vector.tensor_copy(iota128f[:], iota128u[:])
            nc.vector.tensor_scalar(ident[:], iota128f[:], idiagf[:, 0:1], None,
                                    op0=OP.is_equal)

            for b2 in range(2):
                rawBN = spool.tile([128, N * F], F32, name=f"rawBN{b2}", tag="rawbn")
                nc.sync.dma_start(
                    rawBN[:],
                    raw_ext.ap().rearrange("b n f -> b (n f)")[b2 * 128:(b2 + 1) * 128, :])
                for f in range(F):
                    nc.vector.tensor_copy(rawT[:, b2, f, :], rawBN[:, f:N * F:F])
                    # n-interleaved staging: row (b2*128+bp) = chunk
                    # (b2*128+bp)//8, lane (b2*128+bp)%8; col' = c*1600 + n*8 + lane
                    for c in range(16):
                        nc.sync.dma_start(
                            rawxy_dram[f, (b2 * 16 + c) * CHUNK:(b2 * 16 + c + 1) * CHUNK]
                            .rearrange("(n b) -> b n", b=8),
                            rawT[c * 8:(c + 1) * 8, b2, f, :])

            def transpose_into(dst_ap, src_ext_ap, rows):
                tmp = spool.tile([128, 128], F32, name=f"wtmp{transpose_into.k}", tag="wtmp")
                transpose_into.k += 1
                cols = src_ext_ap.shape[1]
                nc.sync.dma_start(tmp[:rows, 0:cols], src_ext_ap)
                tps = ppool.tile([128, GS], F32,
                                 name=f"wtps{transpose_into.k}", tag="attps")
                nc.tensor.transpose(tps[0:cols, 0:rows], tmp[:rows, 0:cols], ident[:])
                nc.vector.tensor_copy(dst_ap, tps[0:cols, 0:rows])
            transpose_into.k = 0

            transpose_into(WseT[:, :], Wse_ext.ap(), H)
            transpose_into(WdeT[:, :], Wde_ext.ap(), H)
            for k in range(3):
                transpose_into(WihT[:, k * H:(k + 1) * H],
                               Wih_ext.ap()[k * H:(k + 1) * H, :], H)
                transpose_into(WhhT[:, k * H:(k + 1) * H],
                               Whh_ext.ap()[k * H:(k + 1) * H, :], H)
            transpose_into(W1T[:, :], W1_ext.ap(), H)
            transpose_into(W2T[:, :], W2_ext.ap(), H)

            def col128(dst, src1d):
                nc.sync.dma_start(dst, src1d.rearrange("(h o) -> h o", o=1))

            col128(v_sb[:], v_ext.ap())
            nc.gpsimd.memset(v32[:], 0.0)
            nc.vector.tensor_copy(v32[:, 0:1], v_sb[:])
            col128(bse_c[:], bse_ext.ap())
            col128(bde_c[:], bde_ext.ap())
            bih_t = cpool.tile([H, 3], F32, name="bih_t")
            bhh_t = cpool.tile([H, 3], F32, name="bhh_t")
            for k in range(3):
                col128(bih_t[:, k:k + 1], bih_ext.ap()[k * H:(k + 1) * H])
                col128(bhh_t[:, k:k + 1], bhh_ext.ap()[k * H:(k + 1) * H])
            # fold W_ih @ b_de into gate biases (b_de then drops out of emb):
            #   b_r/b_z get the 0.5 scale for the tanh-form sigmoid.
            wbd_ps = ppool1.tile([H, 3], F32, name="wbd_ps", tag="gru0")
            for k in range(3):
                nc.tensor.matmul(wbd_ps[:, k:k + 1], WihT[:, k * H:(k + 1) * H],
                                 bde_c[:, 0:1], start=True, stop=True)
            wbd = cpool.tile([H, 3], F32, name="wbd")
            nc.vector.tensor_copy(wbd[:], wbd_ps[:])
            nc.vector.tensor_tensor(b_r[:], bih_t[:, 0:1], bhh_t[:, 0:1], op=OP.add)
            nc.vector.tensor_tensor(b_r[:], b_r[:], wbd[:, 0:1], op=OP.add)
            nc.vector.tensor_scalar(b_r[:], b_r[:], 0.5, None, op0=OP.mult)
            nc.vector.tensor_tensor(b_z[:], bih_t[:, 1:2], bhh_t[:, 1:2], op=OP.add)
            nc.vector.tensor_tensor(b_z[:], b_z[:], wbd[:, 1:2], op=OP.add)
            nc.vector.tensor_scalar(b_z[:], b_z[:], 0.5, None, op0=OP.mult)
            nc.vector.tensor_tensor(b_in[:], bih_t[:, 2:3], wbd[:, 2:3], op=OP.add)
            nc.vector.tensor_copy(b_hn[:], bhh_t[:, 2:3])

            nc.sync.dma_start(fi_sb[:], fi_ext.ap().rearrange("a b f -> f (a b)"))
            for i in range(2):
                nc.vector.tensor_copy(
                    dec_t[i][:].rearrange("f (o b) -> f o b", o=1),
                    fi_sb[:, 0:1].broadcast_to([F, 1, 128]))
                nc.gpsimd.memset(h_t[i][:], 0.0)
            nc.gpsimd.memset(maskadd[:], 0.0)
            nc.gpsimd.memset(maskadd[:, :, 0:1], NEG)

            for ci, (st, sz) in enumerate(chunks):
                rc = spool.tile([F, CHUNK], F32, name=f"rc{ci}", tag="rcring")
                nc.sync.dma_start(rc[:], rawxy_dram[:, st:st + sz])
                for g in range(4):
                    psA = ppool1.tile([H, GS], F32, name=f"psA{ci}_{g}", tag="gru0")
                    nc.tensor.matmul(psA[:], WseT[:, :], rc[:, g * GS:(g + 1) * GS],
                                     start=True, stop=True)
                    feat = spool.tile([H, GS], F32, name=f"feat{ci}_{g}", tag="featring")
                    nc.scalar.activation(feat[:], psA[:], AF.Identity, bias=bse_c[:, 0:1])
                    psB = ppool1.tile([H, GS], F32, name=f"psB{ci}_{g}", tag="gru1")
                    nc.tensor.matmul(psB[:], W1T[:], feat[:], start=True, stop=True)
                    if ci < NRES:
                        nc.vector.tensor_copy(
                            fp_res[:, st + g * GS:st + (g + 1) * GS], psB[:])
                    else:
                        fpo = spool.tile([H, GS], F32, name=f"fpo{ci}_{g}", tag="fpobuf")
                        nc.vector.tensor_copy(fpo[:], psB[:])
                        nc.sync.dma_start(fp_dram[:, st + g * GS:st + (g + 1) * GS],
                                          fpo[:])

            # ================= DECODE LOOP (telescoped halves) =================
            # Invariant at body entry (step t):
            #   h_t[0] = h_{t+1}[:,0:128] (already advanced), q_t[0] = q for
            #   sweep half0 of step t; dec_t[1] = decoder input half1 for
            #   step t's GRU-half1; h_t[1] = h_t[:,128:256] (not yet advanced).
            def gru_half(hf):
                emb_ps = ppool1.tile([H, 128], F32, name=f"emb_ps{hf}", tag=f"gq{hf}")
                nc.tensor.matmul(emb_ps[:], WdeT[:], dec_t[hf][:], start=True, stop=True)
                nc.vector.tensor_copy(emb_sb[hf][:], emb_ps[:])

                r_ps = ppool1.tile([H, 128], F32, name=f"r_ps{hf}", tag=f"gru{hf}")
                nc.tensor.matmul(r_ps[:], WihT[:, 0:H], emb_sb[hf][:], start=True, stop=False)
                nc.tensor.matmul(r_ps[:], WhhT[:, 0:H], h_t[hf][:], start=False, stop=True)
                nc.scalar.activation(r_col[hf][:], r_ps[:], AF.Tanh, bias=b_r[:, 0:1], scale=0.5)
                nc.vector.tensor_scalar(r_col[hf][:], r_col[hf][:], 0.5, 0.5, op0=OP.mult, op1=OP.add)

                z_ps = ppool1.tile([H, 128], F32, name=f"z_ps{hf}", tag=f"gru{hf}")
                nc.tensor.matmul(z_ps[:], WihT[:, H:2 * H], emb_sb[hf][:], start=True, stop=False)
                nc.tensor.matmul(z_ps[:], WhhT[:, H:2 * H], h_t[hf][:], start=False, stop=True)
                nc.scalar.activation(z_col[hf][:], z_ps[:], AF.Tanh, bias=b_z[:, 0:1], scale=0.5)
                nc.vector.tensor_scalar(z_col[hf][:], z_col[hf][:], 0.5, 0.5, op0=OP.mult, op1=OP.add)

                hn_ps = ppool1.tile([H, 128], F32, name=f"hn_ps{hf}", tag=f"gru{hf}")
                nc.tensor.matmul(hn_ps[:], WhhT[:, 2 * H:3 * H], h_t[hf][:], start=True, stop=True)
                nc.scalar.activation(hn_sb[hf][:], hn_ps[:], AF.Identity, bias=b_hn[:, 0:1])

                in_ps = ppool1.tile([H, 128], F32, name=f"in_ps{hf}", tag=f"gru{hf}")
                nc.tensor.matmul(in_ps[:], WihT[:, 2 * H:3 * H], emb_sb[hf][:], start=True, stop=True)

                nc.vector.tensor_tensor(rhn[hf][:], r_col[hf][:], hn_sb[hf][:], op=OP.mult)
                nc.vector.tensor_tensor(npre[hf][:], in_ps[:], rhn[hf][:], op=OP.add)
                nc.scalar.activation(n_col[hf][:], npre[hf][:], AF.Tanh, bias=b_in[:, 0:1])

                nc.vector.tensor_scalar(omz[hf][:], z_col[hf][:], -1.0, 1.0, op0=OP.mult, op1=OP.add)
                nc.vector.tensor_tensor(a1[hf][:], omz[hf][:], n_col[hf][:], op=OP.mult)
                nc.vector.tensor_tensor(a2[hf][:], z_col[hf][:], h_t[hf][:], op=OP.mult)
                nc.vector.tensor_tensor(h_t[hf][:], a1[hf][:], a2[hf][:], op=OP.add)

                q_ps = ppool1.tile([H, 128], F32, name=f"q_ps{hf}", tag=f"gq{hf}")
                nc.tensor.matmul(q_ps[:], W2T[:], h_t[hf][:], start=True, stop=True)
                nc.vector.tensor_copy(q_t[hf][:], q_ps[:])

            # prologue: advance half0 once (step 0) so the invariant holds
            gru_half(0)

            with tc.For_i(0, T_steps) as iv:
                dec_ps_t = [None]

                def tail_b2(b2):
                    nc.vector.tensor_tensor(m_sb[:, b2, :], att_sb[:, b2, :],
                                            maskadd[:, b2, :], op=OP.add)
                    nc.vector.max(mx8[b2][:], m_sb[:, b2, :])
                    nc.vector.max_index(ix8[b2][:], mx8[b2][:], m_sb[:, b2, :])
                    nc.vector.tensor_copy(idxf[b2][:], ix8[b2][:, 0:1])
                    nc.vector.tensor_copy(idxbuf[b2][:, ds(iv, 1)], idxf[b2][:])
                    nc.vector.tensor_scalar(negmax[b2][:], mx8[b2][:, 0:1], -1.0, None,
                                            op0=OP.mult)
                    nc.scalar.activation(m_sb[:, b2, :], m_sb[:, b2, :], AF.Exp,
                                         bias=negmax[b2][:, 0:1],
                                         accum_out=sexpbuf[b2][:, ds(iv, 1)])
                    oh = oh_t[b2]
                    nc.vector.tensor_scalar(oh[:], iota_f[:], idxf[b2][:, 0:1], None,
                                            op0=OP.is_equal)
                    for f in range(F):
                        nc.vector.tensor_tensor(gprod[b2][:, f, :], rawT[:, b2, f, :],
                                                oh[:], op=OP.mult)
                    nc.vector.reduce_sum(dec_b[b2][:], gprod[b2][:],
                                         axis=mybir.AxisListType.X)
                    nc.vector.tensor_scalar(oh[:], oh[:], NEG, None, op0=OP.mult)
                    nc.vector.tensor_tensor(maskadd[:, b2, :], maskadd[:, b2, :], oh[:],
                                            op=OP.add)
                    if dec_ps_t[0] is None:
                        dec_ps_t[0] = ppool1.tile([F, B], F32, name="dec_ps", tag="decps")
                    nc.tensor.transpose(dec_ps_t[0][:, b2 * 128:(b2 + 1) * 128],
                                        dec_b[b2][:], ident[:])
                    nc.vector.tensor_copy(dec_t[b2][:], dec_ps_t[0][:, b2 * 128:(b2 + 1) * 128])

                # phase A: advance half1 (concurrent with sweep half0)
                gru_half(1)

                # ---- attention sweep (paired chunks) ----
                zcp_t = {}
                for ci, (st, sz) in enumerate(chunks):
                    nb = sz // N  # 8
                    b0 = st // N
                    hf = 0 if ci < 16 else 1
                    qv = q_t[hf][:, (b0 % 128):(b0 % 128) + nb]
                    pi = ci // 2   # pair index
                    half = ci % 2
                    if half == 0:
                        zcp_t[pi] = zpool.tile([H, 2, CHUNK], F32,
                                               name=f"zcp{pi}", tag="zring")
                    zcp = zcp_t[pi]
                    if ci < NRES:
                        fpv = fp_res[:, st:st + sz]
                    else:
                        ring = fpool.tile([H, CHUNK], F32, name=f"ring{ci}", tag="fpring")
                        nc.sync.dma_start(ring[:], fp_dram[:, st:st + sz])
                        fpv = ring[:]
                    eng = nc.vector if ci in dve_set else nc.gpsimd
                    eng.tensor_tensor(
                        zcp[:, half, :].rearrange("h (n b) -> h n b", b=nb),
                        fpv.rearrange("h (n b) -> h n b", b=nb),
                        qv.rearrange("h (o b) -> h o b", o=1).broadcast_to([H, N, nb]),
                        op=OP.add)
                    if half == 1:
                        nc.scalar.activation(zcp[:].rearrange("h a b -> h (a b)"),
                                             zcp[:].rearrange("h a b -> h (a b)"),
                                             AF.Tanh)
                    else:
                        continue
                    for hh in range(2):
                        cj = pi * 2 + hh
                        stj = chunks[cj][0]
                        bj = stj // N
                        aps = ppool.tile([128, GS], F32, name=f"aps{cj}", tag="attps")
                        zv = zcp[:, hh, :].rearrange("h (n b) -> h b n", b=nb)
                        for j in range(4):
                            nc.tensor.matmul(
                                aps[32 * j:32 * (j + 1), :],
                                v32[:, :], zv[:, 2 * j:2 * j + 2, :],
                                start=True, stop=True, tile_position=(0, 32 * j),
                                skip_group_check=True)
                        astg = apool.tile([128, GS], F32, name=f"astg{cj}", tag="astg")
                        nc.vector.tensor_copy(astg[:], aps[:])
                        nc.sync.dma_start(
                            att_sb[bj % 128:bj % 128 + nb, bj // 128, :],
                            astg[0:128:32, :].rearrange("p (b n) -> p b n", n=N))
                        if cj == 15:
                            tail_b2(0)
                            # half0 GRU for the NEXT step overlaps sweep half1
                            gru_half(0)
                tail_b2(1)

            # ================= EPILOGUE =================
            for b2 in range(2):
                nc.scalar.activation(sexpbuf[b2][:], sexpbuf[b2][:], AF.Ln)
                nc.vector.tensor_scalar(sexpbuf[b2][:], sexpbuf[b2][:], -1.0, None,
                                        op0=OP.mult)
                nc.sync.dma_start(out_ext.ap()[0, b2 * 128:(b2 + 1) * 128, :], idxbuf[b2][:])
                nc.sync.dma_start(out_ext.ap()[1, b2 * 128:(b2 + 1) * 128, :], sexpbuf[b2][:])

    nc.finalize()
    return nc


def shard_inputs(inputs: dict) -> list[dict]:
    maps = []
    for c in range(8):
        m = {}
        for k, a in inputs.items():
            a = np.ascontiguousarray(np.asarray(a, dtype=np.float32))
            if k == "raw_features":
                m[k] = np.ascontiguousarray(a[c * B:(c + 1) * B])
            else:
                m[k] = a
        maps.append(m)
    return maps


def unshard_output(results: list[dict], T_steps=N - 1) -> np.ndarray:
    idx = np.concatenate([r["out"][0] for r in results], axis=0)
    logp = np.concatenate([r["out"][1] for r in results], axis=0)
    return np.stack([idx, logp])


_NC_CACHE = {}


def kernel(**inputs):
    """Full-batch entry: shard B=2048 across 8 NeuronCores, run, gather."""
    from concourse.bass_utils import run_bass_kernel_spmd
    if "nc" not in _NC_CACHE:
        _NC_CACHE["nc"] = build()
    nc = _NC_CACHE["nc"]
    in_maps = shard_inputs(inputs)
    res = run_bass_kernel_spmd(nc, in_maps, core_ids=list(range(8)))
    return unshard_output(res.results)


# revision 9
# speedup vs baseline: 1.1008x; 1.1008x over previous
"""Bass kernel builder for nn_Actor (pointer-network greedy decoder).

Per-core shard: B=256 rows, N=200 nodes, F=2, H=128, T=199 greedy steps.

Layouts (v3, fp32 sweep + f32r matmuls):
  - col axis n-interleaved per chunk: chunk ci covers 8 b-rows
    (B0=8ci..8ci+7); col = ci*1600 + n*8 + b_local. This makes the
    q-broadcast operand of the attention add packed (last dim = b, stride
    1, bf16) so the DVE runs the add in 2x mode.
  - fp (W1 @ encoded features) stored bf16, ALL 32 chunks SBUF-resident
    (100KB/partition) -> no HBM streaming in the decode loop.
  - zc tiles hold a PAIR of chunks; tanh runs once per pair ([H, 3200]).
  - v-dot: 4 column-tiled bf16 matmuls per chunk (b-pair groups, strided
    rhs view), PSUM out fp32; att assembled (b%128 partitions, b2, n) by
    SBUF->SBUF DMA per chunk.
  - GRU state stays fp32; q is cast to bf16 for the sweep.
"""
import numpy as np
import concourse.bass as bass
import concourse.mybir as mybir
from concourse import bacc
from concourse.tile import TileContext
from concourse.bass import ds

F32 = mybir.dt.float32
F32R = mybir.dt.float32r
BF16 = mybir.dt.bfloat16
U32 = mybir.dt.uint32
AF = mybir.ActivationFunctionType
OP = mybir.AluOpType

B = 256
N = 200
F = 2
H = 128
COLS = B * N
NEG = -1.0e30


def build(T_steps=N - 1, CHUNK=1600, NRES=15, zring=2, fpring=3, dve_k=15):
    assert CHUNK == 1600
    chunks = [(st, CHUNK) for st in range(0, COLS, CHUNK)]
    NCH = len(chunks)  # 32
    GS = CHUNK // 4    # 400 cols per matmul group = 2 b-rows
    # Chunks right after a step/half boundary (0,1,16,17) go to gpsimd: their
    # broadcast-add must not queue behind the argmax tail chain on DVE, or the
    # in-order PE queue stalls per step waiting for chunk 0's zc.
    dve_set = set(np.linspace(2, NCH - 1, dve_k).round().astype(int).tolist())
    dve_set -= {16, 17}

    nc = bacc.Bacc("TRN2", target_bir_lowering=False)

    raw_ext = nc.declare_dram_parameter("raw_features", [B, N, F], F32, isOutput=False)
    fi_ext = nc.declare_dram_parameter("first_input", [1, 1, F], F32, isOutput=False)
    Wse_ext = nc.declare_dram_parameter("W_se", [H, F], F32, isOutput=False)
    bse_ext = nc.declare_dram_parameter("b_se", [H], F32, isOutput=False)
    Wde_ext = nc.declare_dram_parameter("W_de", [H, F], F32, isOutput=False)
    bde_ext = nc.declare_dram_parameter("b_de", [H], F32, isOutput=False)
    Wih_ext = nc.declare_dram_parameter("W_ih", [3 * H, H], F32, isOutput=False)
    Whh_ext = nc.declare_dram_parameter("W_hh", [3 * H, H], F32, isOutput=False)
    bih_ext = nc.declare_dram_parameter("b_ih", [3 * H], F32, isOutput=False)
    bhh_ext = nc.declare_dram_parameter("b_hh", [3 * H], F32, isOutput=False)
    W1_ext = nc.declare_dram_parameter("W1", [H, H], F32, isOutput=False)
    W2_ext = nc.declare_dram_parameter("W2", [H, H], F32, isOutput=False)
    v_ext = nc.declare_dram_parameter("v", [H], F32, isOutput=False)
    out_ext = nc.declare_dram_parameter("out", [2, B, T_steps], F32, isOutput=True)

    with TileContext(nc) as tc:
        with (
            tc.tile_pool(name="const", bufs=1) as cpool,
            tc.tile_pool(name="zr", bufs=zring) as zpool,
            tc.tile_pool(name="fr", bufs=fpring) as fpool,
            tc.tile_pool(name="ar", bufs=3) as apool,
            tc.tile_pool(name="small", bufs=2) as spool,
            tc.tile_pool(name="pbig", bufs=3, space="PSUM") as ppool,
            tc.tile_pool(name="pone", bufs=1, space="PSUM") as ppool1,
            tc.tile_pool(name="dram", bufs=1, space="DRAM") as dpool,
        ):
            # ------------- persistent tiles -------------
            fp_res = cpool.tile([H, NRES * CHUNK], F32, name="fp_res")
            att_sb = cpool.tile([128, 2, N], F32, name="att_sb")
            m_sb = cpool.tile([128, 2, N], F32, name="m_sb")
            maskadd = cpool.tile([128, 2, N], F32, name="maskadd")
            iota_u = cpool.tile([128, N], U32, name="iota_u")
            iota_f = cpool.tile([128, N], F32, name="iota_f")
            rawT = cpool.tile([128, 2, F, N], F32, name="rawT")
            h_t = [cpool.tile([H, 128], F32, name=f"h_t{i}") for i in range(2)]
            dec_t = [cpool.tile([F, 128], F32, name=f"dec_t{i}") for i in range(2)]
            q_t = [cpool.tile([H, 128], F32, name=f"q_t{i}") for i in range(2)]
            idxbuf = [cpool.tile([128, T_steps], F32, name=f"idxbuf{b2}") for b2 in range(2)]
            sexpbuf = [cpool.tile([128, T_steps], F32, name=f"sexpbuf{b2}") for b2 in range(2)]
            idxf = [cpool.tile([128, 1], F32, name=f"idxf{b2}") for b2 in range(2)]
            oh_t = [cpool.tile([128, N], F32, name=f"oh{b2}") for b2 in range(2)]
            gprod = [cpool.tile([128, F, N], F32, name=f"gprod{b2}") for b2 in range(2)]
            dec_b = [cpool.tile([128, F], F32, name=f"dec_b{b2}") for b2 in range(2)]
            mx8 = [cpool.tile([128, 8], F32, name=f"mx8{b2}") for b2 in range(2)]
            ix8 = [cpool.tile([128, 8], U32, name=f"ix8{b2}") for b2 in range(2)]
            negmax = [cpool.tile([128, 1], F32, name=f"negmax{b2}") for b2 in range(2)]

            ident = cpool.tile([128, 128], F32, name="ident")
            WseT = cpool.tile([F, H], F32, name="WseT")
            WdeT = cpool.tile([F, H], F32, name="WdeT")
            WihT = cpool.tile([H, 3 * H], F32, name="WihT")
            WhhT = cpool.tile([H, 3 * H], F32, name="WhhT")
            W1T = cpool.tile([H, H], F32, name="W1T")
            W2T = cpool.tile([H, H], F32, name="W2T")
            v_sb = cpool.tile([H, 1], F32, name="v_sb")
            v32 = cpool.tile([H, 32], F32, name="v32")
            bse_c = cpool.tile([H, 1], F32, name="bse_c")
            bde_c = cpool.tile([H, 1], F32, name="bde_c")
            b_r = cpool.tile([H, 1], F32, name="b_r")
            b_z = cpool.tile([H, 1], F32, name="b_z")
            b_in = cpool.tile([H, 1], F32, name="b_in")
            b_hn = cpool.tile([H, 1], F32, name="b_hn")
            fi_sb = cpool.tile([F, 1], F32, name="fi_sb")

            emb_sb = [cpool.tile([H, 128], F32, name=f"emb_sb{i}") for i in range(2)]
            r_col = [cpool.tile([H, 128], F32, name=f"r_col{i}") for i in range(2)]
            z_col = [cpool.tile([H, 128], F32, name=f"z_col{i}") for i in range(2)]
            hn_sb = [cpool.tile([H, 128], F32, name=f"hn_sb{i}") for i in range(2)]
            rhn = [cpool.tile([H, 128], F32, name=f"rhn{i}") for i in range(2)]
            npre = [cpool.tile([H, 128], F32, name=f"npre{i}") for i in range(2)]
            n_col = [cpool.tile([H, 128], F32, name=f"n_col{i}") for i in range(2)]
            omz = [cpool.tile([H, 128], F32, name=f"omz{i}") for i in range(2)]
            a1 = [cpool.tile([H, 128], F32, name=f"a1_{i}") for i in range(2)]
            a2 = [cpool.tile([H, 128], F32, name=f"a2_{i}") for i in range(2)]

            fp_dram = dpool.tile([H, COLS], F32, name="fp_dram")
            rawxy_dram = dpool.tile([F, COLS], F32, name="rawxy_dram")

            # ================= PRECOMPUTE =================
            nc.gpsimd.iota(iota_u[:], pattern=[[1, N]], base=0, channel_multiplier=0)
            nc.vector.tensor_copy(iota_f[:], iota_u[:])
            idiag = cpool.tile([128, 1], U32, name="idiag")
            nc.gpsimd.iota(idiag[:], pattern=[[0, 1]], base=0, channel_multiplier=1)
            idiagf = cpool.tile([128, 1], F32, name="idiagf")
            nc.vector.tensor_copy(idiagf[:], idiag[:])
            iota128u = cpool.tile([128, 128], U32, name="iota128u")
            nc.gpsimd.iota(iota128u[:], pattern=[[1, 128]], base=0, channel_multiplier=0)
            iota128f = cpool.tile([128, 128], F32, name="iota128f")
            nc.vector.tensor_copy(iota128f[:], iota128u[:])
            nc.vector.tensor_scalar(ident[:], iota128f[:], idiagf[:, 0:1], None,
                                    op0=OP.is_equal)

            for b2 in range(2):
                rawBN = spool.tile([128, N * F], F32, name=f"rawBN{b2}", tag="rawbn")
                nc.sync.dma_start(
                    rawBN[:],
                    raw_ext.ap().rearrange("b n f -> b (n f)")[b2 * 128:(b2 + 1) * 128, :])
                for f in range(F):
                    nc.vector.tensor_copy(rawT[:, b2, f, :], rawBN[:, f:N * F:F])
                    nc.sync.dma_start(
                        rawxy_dram[f, b2 * 128 * N:(b2 + 1) * 128 * N]
                        .rearrange("(bp n) -> bp n", n=N),
                        rawT[:, b2, f, :])

            def transpose_into(dst_ap, src_ext_ap, rows):
                tmp = spool.tile([128, 128], F32, name=f"wtmp{transpose_into.k}", tag="wtmp")
                transpose_into.k += 1
                cols = src_ext_ap.shape[1]
                nc.sync.dma_start(tmp[:rows, 0:cols], src_ext_ap)
                tps = ppool.tile([128, GS], F32,
                                 name=f"wtps{transpose_into.k}", tag="attps")
                nc.tensor.transpose(tps[0:cols, 0:rows], tmp[:rows, 0:cols], ident[:])
                nc.vector.tensor_copy(dst_ap, tps[0:cols, 0:rows])
            transpose_into.k = 0

            transpose_into(WseT[:, :], Wse_ext.ap(), H)
            transpose_into(WdeT[:, :], Wde_ext.ap(), H)
            for k in range(3):
                transpose_into(WihT[:, k * H:(k + 1) * H],
                               Wih_ext.ap()[k * H:(k + 1) * H, :], H)
                transpose_into(WhhT[:, k * H:(k + 1) * H],
                               Whh_ext.ap()[k * H:(k + 1) * H, :], H)
            transpose_into(W1T[:, :], W1_ext.ap(), H)
            transpose_into(W2T[:, :], W2_ext.ap(), H)

            def col128(dst, src1d):
                nc.sync.dma_start(dst, src1d.rearrange("(h o) -> h o", o=1))

            col128(v_sb[:], v_ext.ap())
            nc.gpsimd.memset(v32[:], 0.0)
            nc.vector.tensor_copy(v32[:, 0:1], v_sb[:])
            col128(bse_c[:], bse_ext.ap())
            col128(bde_c[:], bde_ext.ap())
            bih_t = cpool.tile([H, 3], F32, name="bih_t")
            bhh_t = cpool.tile([H, 3], F32, name="bhh_t")
            for k in range(3):
                col128(bih_t[:, k:k + 1], bih_ext.ap()[k * H:(k + 1) * H])
                col128(bhh_t[:, k:k + 1], bhh_ext.ap()[k * H:(k + 1) * H])
            # fold W_ih @ b_de into gate biases (b_de then drops out of emb):
            #   b_r/b_z get the 0.5 scale for the tanh-form sigmoid.
            wbd_ps = ppool1.tile([H, 3], F32, name="wbd_ps", tag="gru0")
            for k in range(3):
                nc.tensor.matmul(wbd_ps[:, k:k + 1], WihT[:, k * H:(k + 1) * H],
                                 bde_c[:, 0:1], start=True, stop=True)
            wbd = cpool.tile([H, 3], F32, name="wbd")
            nc.vector.tensor_copy(wbd[:], wbd_ps[:])
            nc.vector.tensor_tensor(b_r[:], bih_t[:, 0:1], bhh_t[:, 0:1], op=OP.add)
            nc.vector.tensor_tensor(b_r[:], b_r[:], wbd[:, 0:1], op=OP.add)
            nc.vector.tensor_scalar(b_r[:], b_r[:], 0.5, None, op0=OP.mult)
            nc.vector.tensor_tensor(b_z[:], bih_t[:, 1:2], bhh_t[:, 1:2], op=OP.add)
            nc.vector.tensor_tensor(b_z[:], b_z[:], wbd[:, 1:2], op=OP.add)
            nc.vector.tensor_scalar(b_z[:], b_z[:], 0.5, None, op0=OP.mult)
            nc.vector.tensor_tensor(b_in[:], bih_t[:, 2:3], wbd[:, 2:3], op=OP.add)
            nc.vector.tensor_copy(b_hn[:], bhh_t[:, 2:3])

            nc.sync.dma_start(fi_sb[:], fi_ext.ap().rearrange("a b f -> f (a b)"))
            for i in range(2):
                nc.vector.tensor_copy(
                    dec_t[i][:].rearrange("f (o b) -> f o b", o=1),
                    fi_sb[:, 0:1].broadcast_to([F, 1, 128]))
                nc.gpsimd.memset(h_t[i][:], 0.0)
            nc.gpsimd.memset(maskadd[:], 0.0)
            nc.gpsimd.memset(maskadd[:, :, 0:1], NEG)

            for ci, (st, sz) in enumerate(chunks):
                rc = spool.tile([F, CHUNK], F32, name=f"rc{ci}", tag="rcring")
                nc.sync.dma_start(rc[:], rawxy_dram[:, st:st + sz])
                for g in range(4):
                    psA = ppool1.tile([H, GS], F32, name=f"psA{ci}_{g}", tag="gru0")
                    nc.tensor.matmul(psA[:], WseT[:, :], rc[:, g * GS:(g + 1) * GS],
                                     start=True, stop=True)
                    feat = spool.tile([H, GS], F32, name=f"feat{ci}_{g}", tag="featring")
                    nc.scalar.activation(feat[:], psA[:], AF.Identity, bias=bse_c[:, 0:1])
                    psB = ppool1.tile([H, GS], F32, name=f"psB{ci}_{g}", tag="gru1")
                    nc.tensor.matmul(psB[:], W1T[:], feat[:], start=True, stop=True)
                    if ci < NRES:
                        nc.vector.tensor_copy(
                            fp_res[:, st + g * GS:st + (g + 1) * GS], psB[:])
                    else:
                        fpo = spool.tile([H, GS], F32, name=f"fpo{ci}_{g}", tag="fpobuf")
                        nc.vector.tensor_copy(fpo[:], psB[:])
                        nc.sync.dma_start(fp_dram[:, st + g * GS:st + (g + 1) * GS],
                                          fpo[:])

            # ================= DECODE LOOP (telescoped halves) =================
            # Invariant at body entry (step t):
            #   h_t[0] = h_{t+1}[:,0:128] (already advanced), q_t[0] = q for
            #   sweep half0 of step t; dec_t[1] = decoder input half1 for
            #   step t's GRU-half1; h_t[1] = h_t[:,128:256] (not yet advanced).
            def gru_half(hf):
                emb_ps = ppool1.tile([H, 128], F32, name=f"emb_ps{hf}", tag=f"gq{hf}")
                nc.tensor.matmul(emb_ps[:], WdeT[:], dec_t[hf][:], start=True, stop=True)
                nc.vector.tensor_copy(emb_sb[hf][:], emb_ps[:])

                r_ps = ppool1.tile([H, 128], F32, name=f"r_ps{hf}", tag=f"gru{hf}")
                nc.tensor.matmul(r_ps[:], WihT[:, 0:H], emb_sb[hf][:], start=True, stop=False)
                nc.tensor.matmul(r_ps[:], WhhT[:, 0:H], h_t[hf][:], start=False, stop=True)
                nc.scalar.activation(r_col[hf][:], r_ps[:], AF.Tanh, bias=b_r[:, 0:1], scale=0.5)
                nc.vector.tensor_scalar(r_col[hf][:], r_col[hf][:], 0.5, 0.5, op0=OP.mult, op1=OP.add)

                z_ps = ppool1.tile([H, 128], F32, name=f"z_ps{hf}", tag=f"gru{hf}")
                nc.tensor.matmul(z_ps[:], WihT[:, H:2 * H], emb_sb[hf][:], start=True, stop=False)
                nc.tensor.matmul(z_ps[:], WhhT[:, H:2 * H], h_t[hf][:], start=False, stop=True)
                nc.scalar.activation(z_col[hf][:], z_ps[:], AF.Tanh, bias=b_z[:, 0:1], scale=0.5)
                nc.vector.tensor_scalar(z_col[hf][:], z_col[hf][:], 0.5, 0.5, op0=OP.mult, op1=OP.add)

                hn_ps = ppool1.tile([H, 128], F32, name=f"hn_ps{hf}", tag=f"gru{hf}")
                nc.tensor.matmul(hn_ps[:], WhhT[:, 2 * H:3 * H], h_t[hf][:], start=True, stop=True)
                nc.scalar.activation(hn_sb[hf][:], hn_ps[:], AF.Identity, bias=b_hn[:, 0:1])

                in_ps = ppool1.tile([H, 128], F32, name=f"in_ps{hf}", tag=f"gru{hf}")
                nc.tensor.matmul(in_ps[:], WihT[:, 2 * H:3 * H], emb_sb[hf][:], start=True, stop=True)

                nc.vector.tensor_tensor(rhn[hf][:], r_col[hf][:], hn_sb[hf][:], op=OP.mult)
                nc.vector.tensor_tensor(npre[hf][:], in_ps[:], rhn[hf][:], op=OP.add)
                nc.scalar.activation(n_col[hf][:], npre[hf][:], AF.Tanh, bias=b_in[:, 0:1])

                nc.vector.tensor_scalar(omz[hf][:], z_col[hf][:], -1.0, 1.0, op0=OP.mult, op1=OP.add)
                nc.vector.tensor_tensor(a1[hf][:], omz[hf][:], n_col[hf][:], op=OP.mult)
                nc.vector.tensor_tensor(a2[hf][:], z_col[hf][:], h_t[hf][:], op=OP.mult)
                nc.vector.tensor_tensor(h_t[hf][:], a1[hf][:], a2[hf][:], op=OP.add)

                q_ps = ppool1.tile([H, 128], F32, name=f"q_ps{hf}", tag=f"gq{hf}")
                nc.tensor.matmul(q_ps[:], W2T[:], h_t[hf][:], start=True, stop=True)
                nc.vector.tensor_copy(q_t[hf][:], q_ps[:])

            # prologue: advance half0 once (step 0) so the invariant holds
            gru_half(0)

            with tc.For_i(0, T_steps) as iv:
                dec_ps_t = [None]

                def tail_b2(b2):
                    nc.vector.tensor_tensor(m_sb[:, b2, :], att_sb[:, b2, :],
                                            maskadd[:, b2, :], op=OP.add)
                    nc.vector.max(mx8[b2][:], m_sb[:, b2, :])
                    nc.vector.max_index(ix8[b2][:], mx8[b2][:], m_sb[:, b2, :])
                    nc.vector.tensor_copy(idxf[b2][:], ix8[b2][:, 0:1])
                    nc.vector.tensor_copy(idxbuf[b2][:, ds(iv, 1)], idxf[b2][:])
                    nc.vector.tensor_scalar(negmax[b2][:], mx8[b2][:, 0:1], -1.0, None,
                                            op0=OP.mult)
                    nc.scalar.activation(m_sb[:, b2, :], m_sb[:, b2, :], AF.Exp,
                                         bias=negmax[b2][:, 0:1],
                                         accum_out=sexpbuf[b2][:, ds(iv, 1)])
                    oh = oh_t[b2]
                    nc.vector.tensor_scalar(oh[:], iota_f[:], idxf[b2][:, 0:1], None,
                                            op0=OP.is_equal)
                    for f in range(F):
                        nc.vector.tensor_tensor(gprod[b2][:, f, :], rawT[:, b2, f, :],
                                                oh[:], op=OP.mult)
                    nc.vector.reduce_sum(dec_b[b2][:], gprod[b2][:],
                                         axis=mybir.AxisListType.X)
                    nc.vector.tensor_scalar(oh[:], oh[:], NEG, None, op0=OP.mult)
                    nc.vector.tensor_tensor(maskadd[:, b2, :], maskadd[:, b2, :], oh[:],
                                            op=OP.add)
                    if dec_ps_t[0] is None:
                        dec_ps_t[0] = ppool1.tile([F, B], F32, name="dec_ps", tag="decps")
                    nc.tensor.transpose(dec_ps_t[0][:, b2 * 128:(b2 + 1) * 128],
                                        dec_b[b2][:], ident[:])
                    nc.vector.tensor_copy(dec_t[b2][:], dec_ps_t[0][:, b2 * 128:(b2 + 1) * 128])

                # phase A: advance half1 (concurrent with sweep half0)
                gru_half(1)

                # ---- attention sweep (paired chunks) ----
                zcp_t = {}
                for ci, (st, sz) in enumerate(chunks):
                    nb = sz // N  # 8
                    b0 = st // N
                    hf = 0 if ci < 16 else 1
                    qv = q_t[hf][:, (b0 % 128):(b0 % 128) + nb]
                    pi = ci // 2   # pair index
                    half = ci % 2
                    if half == 0:
                        zcp_t[pi] = zpool.tile([H, 2, CHUNK], F32,
                                               name=f"zcp{pi}", tag="zring")
                    zcp = zcp_t[pi]
                    if ci < NRES:
                        fpv = fp_res[:, st:st + sz]
                    else:
                        ring = fpool.tile([H, CHUNK], F32, name=f"ring{ci}", tag="fpring")
                        nc.sync.dma_start(ring[:], fp_dram[:, st:st + sz])
                        fpv = ring[:]
                    eng = nc.vector if ci in dve_set else nc.gpsimd
                    eng.tensor_tensor(
                        zcp[:, half, :].rearrange("h (b n) -> h b n", n=N),
                        fpv.rearrange("h (b n) -> h b n", n=N),
                        qv.broadcast_to([H, nb, N]),
                        op=OP.add)
                    if half == 1:
                        nc.scalar.activation(zcp[:].rearrange("h a b -> h (a b)"),
                                             zcp[:].rearrange("h a b -> h (a b)"),
                                             AF.Tanh)
                    else:
                        continue
                    for hh in range(2):
                        cj = pi * 2 + hh
                        stj = chunks[cj][0]
                        bj = stj // N
                        aps = ppool.tile([128, GS], F32, name=f"aps{cj}", tag="attps")
                        zv = zcp[:, hh, :]
                        for j in range(4):
                            nc.tensor.matmul(
                                aps[32 * j:32 * (j + 1), :],
                                v32[:, :], zv[:, j * GS:(j + 1) * GS],
                                start=True, stop=True, tile_position=(0, 32 * j),
                                skip_group_check=True)
                        astg = apool.tile([128, GS], F32, name=f"astg{cj}", tag="astg")
                        nc.vector.tensor_copy(astg[:], aps[:])
                        nc.sync.dma_start(
                            att_sb[bj % 128:bj % 128 + nb, bj // 128, :],
                            astg[0:128:32, :].rearrange("p (b n) -> p b n", n=N))
                        if cj == 15:
                            tail_b2(0)
                            # half0 GRU for the NEXT step overlaps sweep half1
                            gru_half(0)
                tail_b2(1)

            # ================= EPILOGUE =================
            for b2 in range(2):
                nc.scalar.activation(sexpbuf[b2][:], sexpbuf[b2][:], AF.Ln)
                nc.vector.tensor_scalar(sexpbuf[b2][:], sexpbuf[b2][:], -1.0, None,
                                        op0=OP.mult)
                nc.sync.dma_start(out_ext.ap()[0, b2 * 128:(b2 + 1) * 128, :], idxbuf[b2][:])
                nc.sync.dma_start(out_ext.ap()[1, b2 * 128:(b2 + 1) * 128, :], sexpbuf[b2][:])

    nc.finalize()
    return nc


def shard_inputs(inputs: dict) -> list[dict]:
    maps = []
    for c in range(8):
        m = {}
        for k, a in inputs.items():
            a = np.ascontiguousarray(np.asarray(a, dtype=np.float32))
            if k == "raw_features":
                m[k] = np.ascontiguousarray(a[c * B:(c + 1) * B])
            else:
                m[k] = a
        maps.append(m)
    return maps


def unshard_output(results: list[dict], T_steps=N - 1) -> np.ndarray:
    idx = np.concatenate([r["out"][0] for r in results], axis=0)
    logp = np.concatenate([r["out"][1] for r in results], axis=0)
    return np.stack([idx, logp])


_NC_CACHE = {}


def kernel(**inputs):
    """Full-batch entry: shard B=2048 across 8 NeuronCores, run, gather."""
    from concourse.bass_utils import run_bass_kernel_spmd
    if "nc" not in _NC_CACHE:
        _NC_CACHE["nc"] = build()
    nc = _NC_CACHE["nc"]
    in_maps = shard_inputs(inputs)
    res = run_bass_kernel_spmd(nc, in_maps, core_ids=list(range(8)))
    return unshard_output(res.results)


# revision 15
# speedup vs baseline: 1.2994x; 1.1804x over previous
"""Bass kernel builder for nn_Actor (pointer-network greedy decoder).

Per-core shard: B=256 rows, N=200 nodes, F=2, H=128, T=199 greedy steps.

Layouts:
  - col axis: col = b*200 + n (b-major), 51200 cols, CHUNK=1600 (8 b-rows),
    32 chunks; each chunk's v-dot uses 4 column-tiled fp32 matmuls with
    gs=400 (2 b-rows per column group).
  - feat_proj fp: (h=128p, col); NRES chunks SBUF-resident, rest streamed.
  - att assembled (b%128 partitions, b2, n) by SBUF->SBUF DMA per chunk.
  - GRU state h_col/q_col/dec_col: (feature partitions, b=256 cols).
"""
import numpy as np
import concourse.bass as bass
import concourse.mybir as mybir
from concourse import bacc
from concourse.tile import TileContext
from concourse.bass import ds

F32 = mybir.dt.float32
U32 = mybir.dt.uint32
AF = mybir.ActivationFunctionType
OP = mybir.AluOpType

B = 256
N = 200
F = 2
H = 128
COLS = B * N
NEG = -1.0e30


def build(T_steps=N - 1, CHUNK=1600, NRES=16, zring=5, fpring=4, dve_k=14):
    assert CHUNK == 1600
    chunks = [(st, CHUNK) for st in range(0, COLS, CHUNK)]
    NCH = len(chunks)  # 32
    GS = CHUNK // 4    # 400 cols per column-group = 2 b-rows
    # Chunks right after a step/half boundary (0,1,16,17) go to gpsimd: their
    # broadcast-add must not queue behind the argmax tail chain on DVE, or the
    # in-order PE queue stalls ~5us per step waiting for chunk 0's zc.
    dve_set = set(np.linspace(2, NCH - 1, dve_k).round().astype(int).tolist())
    dve_set -= {16, 17}

    nc = bacc.Bacc("TRN2", target_bir_lowering=False)

    raw_ext = nc.declare_dram_parameter("raw_features", [B, N, F], F32, isOutput=False)
    fi_ext = nc.declare_dram_parameter("first_input", [1, 1, F], F32, isOutput=False)
    Wse_ext = nc.declare_dram_parameter("W_se", [H, F], F32, isOutput=False)
    bse_ext = nc.declare_dram_parameter("b_se", [H], F32, isOutput=False)
    Wde_ext = nc.declare_dram_parameter("W_de", [H, F], F32, isOutput=False)
    bde_ext = nc.declare_dram_parameter("b_de", [H], F32, isOutput=False)
    Wih_ext = nc.declare_dram_parameter("W_ih", [3 * H, H], F32, isOutput=False)
    Whh_ext = nc.declare_dram_parameter("W_hh", [3 * H, H], F32, isOutput=False)
    bih_ext = nc.declare_dram_parameter("b_ih", [3 * H], F32, isOutput=False)
    bhh_ext = nc.declare_dram_parameter("b_hh", [3 * H], F32, isOutput=False)
    W1_ext = nc.declare_dram_parameter("W1", [H, H], F32, isOutput=False)
    W2_ext = nc.declare_dram_parameter("W2", [H, H], F32, isOutput=False)
    v_ext = nc.declare_dram_parameter("v", [H], F32, isOutput=False)
    out_ext = nc.declare_dram_parameter("out", [2, B, T_steps], F32, isOutput=True)

    with TileContext(nc) as tc:
        with (
            tc.tile_pool(name="const", bufs=1) as cpool,
            tc.tile_pool(name="zr", bufs=zring) as zpool,
            tc.tile_pool(name="fr", bufs=fpring) as fpool,
            tc.tile_pool(name="ar", bufs=3) as apool,
            tc.tile_pool(name="small", bufs=2) as spool,
            tc.tile_pool(name="pbig", bufs=3, space="PSUM") as ppool,
            tc.tile_pool(name="pone", bufs=1, space="PSUM") as ppool1,
            tc.tile_pool(name="dram", bufs=1, space="DRAM") as dpool,
        ):
            # ------------- persistent tiles -------------
            fp_res = cpool.tile([H, NRES * CHUNK], F32, name="fp_res")
            att_sb = cpool.tile([128, 2, N], F32, name="att_sb")
            m_sb = cpool.tile([128, 2, N], F32, name="m_sb")
            maskadd = cpool.tile([128, 2, N], F32, name="maskadd")
            iota_u = cpool.tile([128, N], U32, name="iota_u")
            iota_f = cpool.tile([128, N], F32, name="iota_f")
            rawT = cpool.tile([128, 2, F, N], F32, name="rawT")
            h_t = [cpool.tile([H, 128], F32, name=f"h_t{i}") for i in range(2)]
            dec_t = [cpool.tile([F, 128], F32, name=f"dec_t{i}") for i in range(2)]
            q_t = [cpool.tile([H, 128], F32, name=f"q_t{i}") for i in range(2)]
            idxbuf = [cpool.tile([128, T_steps], F32, name=f"idxbuf{b2}") for b2 in range(2)]
            sexpbuf = [cpool.tile([128, T_steps], F32, name=f"sexpbuf{b2}") for b2 in range(2)]
            idxf = [cpool.tile([128, 1], F32, name=f"idxf{b2}") for b2 in range(2)]
            oh_t = [cpool.tile([128, N], F32, name=f"oh{b2}") for b2 in range(2)]
            gprod = [cpool.tile([128, F, N], F32, name=f"gprod{b2}") for b2 in range(2)]
            dec_b = [cpool.tile([128, F], F32, name=f"dec_b{b2}") for b2 in range(2)]
            mx8 = [cpool.tile([128, 8], F32, name=f"mx8{b2}") for b2 in range(2)]
            ix8 = [cpool.tile([128, 8], U32, name=f"ix8{b2}") for b2 in range(2)]
            negmax = [cpool.tile([128, 1], F32, name=f"negmax{b2}") for b2 in range(2)]

            ident = cpool.tile([128, 128], F32, name="ident")
            WseT = cpool.tile([F, H], F32, name="WseT")
            WdeT = cpool.tile([F, H], F32, name="WdeT")
            WihT = cpool.tile([H, 3 * H], F32, name="WihT")
            WhhT = cpool.tile([H, 3 * H], F32, name="WhhT")
            W1T = cpool.tile([H, H], F32, name="W1T")
            W2T = cpool.tile([H, H], F32, name="W2T")
            v_sb = cpool.tile([H, 1], F32, name="v_sb")
            v32 = cpool.tile([H, 32], F32, name="v32")
            bse_c = cpool.tile([H, 1], F32, name="bse_c")
            bde_c = cpool.tile([H, 1], F32, name="bde_c")
            b_r = cpool.tile([H, 1], F32, name="b_r")
            b_z = cpool.tile([H, 1], F32, name="b_z")
            b_in = cpool.tile([H, 1], F32, name="b_in")
            b_hn = cpool.tile([H, 1], F32, name="b_hn")
            fi_sb = cpool.tile([F, 1], F32, name="fi_sb")

            emb_sb = [cpool.tile([H, 128], F32, name=f"emb_sb{i}") for i in range(2)]
            r_col = [cpool.tile([H, 128], F32, name=f"r_col{i}") for i in range(2)]
            z_col = [cpool.tile([H, 128], F32, name=f"z_col{i}") for i in range(2)]
            hn_sb = [cpool.tile([H, 128], F32, name=f"hn_sb{i}") for i in range(2)]
            rhn = [cpool.tile([H, 128], F32, name=f"rhn{i}") for i in range(2)]
            npre = [cpool.tile([H, 128], F32, name=f"npre{i}") for i in range(2)]
            n_col = [cpool.tile([H, 128], F32, name=f"n_col{i}") for i in range(2)]
            omz = [cpool.tile([H, 128], F32, name=f"omz{i}") for i in range(2)]
            a1 = [cpool.tile([H, 128], F32, name=f"a1_{i}") for i in range(2)]
            a2 = [cpool.tile([H, 128], F32, name=f"a2_{i}") for i in range(2)]

            fp_dram = dpool.tile([H, COLS], F32, name="fp_dram")
            rawxy_dram = dpool.tile([F, COLS], F32, name="rawxy_dram")

            # ================= PRECOMPUTE =================
            nc.gpsimd.iota(iota_u[:], pattern=[[1, N]], base=0, channel_multiplier=0)
            nc.vector.tensor_copy(iota_f[:], iota_u[:])
            idiag = cpool.tile([128, 1], U32, name="idiag")
            nc.gpsimd.iota(idiag[:], pattern=[[0, 1]], base=0, channel_multiplier=1)
            idiagf = cpool.tile([128, 1], F32, name="idiagf")
            nc.vector.tensor_copy(idiagf[:], idiag[:])
            iota128u = cpool.tile([128, 128], U32, name="iota128u")
            nc.gpsimd.iota(iota128u[:], pattern=[[1, 128]], base=0, channel_multiplier=0)
            iota128f = cpool.tile([128, 128], F32, name="iota128f")
            nc.vector.tensor_copy(iota128f[:], iota128u[:])
            nc.vector.tensor_scalar(ident[:], iota128f[:], idiagf[:, 0:1], None,
                                    op0=OP.is_equal)

            for b2 in range(2):
                rawBN = zpool.tile([128, N * F], F32, name=f"rawBN{b2}", tag="zring")
                nc.sync.dma_start(
                    rawBN[:],
                    raw_ext.ap().rearrange("b n f -> b (n f)")[b2 * 128:(b2 + 1) * 128, :])
                for f in range(F):
                    nc.vector.tensor_copy(rawT[:, b2, f, :], rawBN[:, f:N * F:F])
                    nc.sync.dma_start(
                        rawxy_dram[f, b2 * 128 * N:(b2 + 1) * 128 * N]
                        .rearrange("(bp n) -> bp n", n=N),
                        rawT[:, b2, f, :])

            def transpose_into(dst_ap, src_ext_ap, rows):
                tmp = spool.tile([128, 128], F32, name=f"wtmp{transpose_into.k}", tag="wtmp")
                transpose_into.k += 1
                cols = src_ext_ap.shape[1]
                nc.sync.dma_start(tmp[:rows, 0:cols], src_ext_ap)
                tps = ppool.tile([128, GS], F32,
                                 name=f"wtps{transpose_into.k}", tag="attps")
                nc.tensor.transpose(tps[0:cols, 0:rows], tmp[:rows, 0:cols], ident[:])
                nc.vector.tensor_copy(dst_ap, tps[0:cols, 0:rows])
            transpose_into.k = 0

            transpose_into(WseT[:, :], Wse_ext.ap(), H)
            transpose_into(WdeT[:, :], Wde_ext.ap(), H)
            for k in range(3):
                transpose_into(WihT[:, k * H:(k + 1) * H],
                               Wih_ext.ap()[k * H:(k + 1) * H, :], H)
                transpose_into(WhhT[:, k * H:(k + 1) * H],
                               Whh_ext.ap()[k * H:(k + 1) * H, :], H)
            transpose_into(W1T[:, :], W1_ext.ap(), H)
            transpose_into(W2T[:, :], W2_ext.ap(), H)

            def col128(dst, src1d):
                nc.sync.dma_start(dst, src1d.rearrange("(h o) -> h o", o=1))

            col128(v_sb[:], v_ext.ap())
            nc.gpsimd.memset(v32[:], 0.0)
            nc.vector.tensor_copy(v32[:, 0:1], v_sb[:])
            col128(bse_c[:], bse_ext.ap())
            col128(bde_c[:], bde_ext.ap())
            bih_t = cpool.tile([H, 3], F32, name="bih_t")
            bhh_t = cpool.tile([H, 3], F32, name="bhh_t")
            for k in range(3):
                col128(bih_t[:, k:k + 1], bih_ext.ap()[k * H:(k + 1) * H])
                col128(bhh_t[:, k:k + 1], bhh_ext.ap()[k * H:(k + 1) * H])
            # fold W_ih @ b_de into gate biases (b_de then drops out of emb):
            #   b_r/b_z get the 0.5 scale for the tanh-form sigmoid.
            wbd_ps = ppool1.tile([H, 3], F32, name="wbd_ps", tag="gru0")
            for k in range(3):
                nc.tensor.matmul(wbd_ps[:, k:k + 1], WihT[:, k * H:(k + 1) * H],
                                 bde_c[:, 0:1], start=True, stop=True)
            wbd = cpool.tile([H, 3], F32, name="wbd")
            nc.vector.tensor_copy(wbd[:], wbd_ps[:])
            nc.vector.tensor_tensor(b_r[:], bih_t[:, 0:1], bhh_t[:, 0:1], op=OP.add)
            nc.vector.tensor_tensor(b_r[:], b_r[:], wbd[:, 0:1], op=OP.add)
            nc.vector.tensor_scalar(b_r[:], b_r[:], 0.5, None, op0=OP.mult)
            nc.vector.tensor_tensor(b_z[:], bih_t[:, 1:2], bhh_t[:, 1:2], op=OP.add)
            nc.vector.tensor_tensor(b_z[:], b_z[:], wbd[:, 1:2], op=OP.add)
            nc.vector.tensor_scalar(b_z[:], b_z[:], 0.5, None, op0=OP.mult)
            nc.vector.tensor_tensor(b_in[:], bih_t[:, 2:3], wbd[:, 2:3], op=OP.add)
            nc.vector.tensor_copy(b_hn[:], bhh_t[:, 2:3])

            nc.sync.dma_start(fi_sb[:], fi_ext.ap().rearrange("a b f -> f (a b)"))
            for i in range(2):
                nc.vector.tensor_copy(
                    dec_t[i][:].rearrange("f (o b) -> f o b", o=1),
                    fi_sb[:, 0:1].broadcast_to([F, 1, 128]))
                nc.gpsimd.memset(h_t[i][:], 0.0)
            nc.gpsimd.memset(maskadd[:], 0.0)
            nc.gpsimd.memset(maskadd[:, :, 0:1], NEG)

            GSp = GS
            for ci, (st, sz) in enumerate(chunks):
                rc = fpool.tile([F, CHUNK], F32, name=f"rc{ci}", tag="fpring")
                nc.sync.dma_start(rc[:], rawxy_dram[:, st:st + sz])
                for g in range(4):
                    psA = ppool1.tile([H, GSp], F32, name=f"psA{ci}_{g}", tag="gru0")
                    nc.tensor.matmul(psA[:], WseT[:, :], rc[:, g * GSp:(g + 1) * GSp],
                                     start=True, stop=True)
                    feat = zpool.tile([H, GSp], F32, name=f"feat{ci}_{g}", tag="zring")
                    nc.scalar.activation(feat[:], psA[:], AF.Identity, bias=bse_c[:, 0:1])
                    psB = ppool1.tile([H, GSp], F32, name=f"psB{ci}_{g}", tag="gru1")
                    nc.tensor.matmul(psB[:], W1T[:, :], feat[:], start=True, stop=True)
                    if ci < NRES:
                        nc.vector.tensor_copy(
                            fp_res[:, ci * CHUNK + g * GSp:ci * CHUNK + (g + 1) * GSp],
                            psB[:])
                    else:
                        fpo = fpool.tile([H, GSp], F32, name=f"fpo{ci}_{g}", tag="fpring")
                        nc.vector.tensor_copy(fpo[:], psB[:])
                        nc.sync.dma_start(fp_dram[:, st + g * GSp:st + (g + 1) * GSp],
                                          fpo[:])

            # ================= DECODE LOOP (telescoped halves) =================
            # Invariant at body entry (step t):
            #   h_t[0] = h_{t+1}[:,0:128] (already advanced), q_t[0] = q for
            #   sweep half0 of step t; dec_t[1] = decoder input half1 for
            #   step t's GRU-half1; h_t[1] = h_t[:,128:256] (not yet advanced).
            def gru_half(hf):
                emb_ps = ppool1.tile([H, 128], F32, name=f"emb_ps{hf}", tag=f"gq{hf}")
                nc.tensor.matmul(emb_ps[:], WdeT[:], dec_t[hf][:], start=True, stop=True)
                nc.vector.tensor_copy(emb_sb[hf][:], emb_ps[:])

                r_ps = ppool1.tile([H, 128], F32, name=f"r_ps{hf}", tag=f"gru{hf}")
                nc.tensor.matmul(r_ps[:], WihT[:, 0:H], emb_sb[hf][:], start=True, stop=False)
                nc.tensor.matmul(r_ps[:], WhhT[:, 0:H], h_t[hf][:], start=False, stop=True)
                nc.scalar.activation(r_col[hf][:], r_ps[:], AF.Tanh, bias=b_r[:, 0:1], scale=0.5)
                nc.vector.tensor_scalar(r_col[hf][:], r_col[hf][:], 0.5, 0.5, op0=OP.mult, op1=OP.add)

                z_ps = ppool1.tile([H, 128], F32, name=f"z_ps{hf}", tag=f"gru{hf}")
                nc.tensor.matmul(z_ps[:], WihT[:, H:2 * H], emb_sb[hf][:], start=True, stop=False)
                nc.tensor.matmul(z_ps[:], WhhT[:, H:2 * H], h_t[hf][:], start=False, stop=True)
                nc.scalar.activation(z_col[hf][:], z_ps[:], AF.Tanh, bias=b_z[:, 0:1], scale=0.5)
                nc.vector.tensor_scalar(z_col[hf][:], z_col[hf][:], 0.5, 0.5, op0=OP.mult, op1=OP.add)

                hn_ps = ppool1.tile([H, 128], F32, name=f"hn_ps{hf}", tag=f"gru{hf}")
                nc.tensor.matmul(hn_ps[:], WhhT[:, 2 * H:3 * H], h_t[hf][:], start=True, stop=True)
                nc.scalar.activation(hn_sb[hf][:], hn_ps[:], AF.Identity, bias=b_hn[:, 0:1])

                in_ps = ppool1.tile([H, 128], F32, name=f"in_ps{hf}", tag=f"gru{hf}")
                nc.tensor.matmul(in_ps[:], WihT[:, 2 * H:3 * H], emb_sb[hf][:], start=True, stop=True)

                nc.vector.tensor_tensor(rhn[hf][:], r_col[hf][:], hn_sb[hf][:], op=OP.mult)
                nc.vector.tensor_tensor(npre[hf][:], in_ps[:], rhn[hf][:], op=OP.add)
                nc.scalar.activation(n_col[hf][:], npre[hf][:], AF.Tanh, bias=b_in[:, 0:1])

                nc.vector.tensor_scalar(omz[hf][:], z_col[hf][:], -1.0, 1.0, op0=OP.mult, op1=OP.add)
                nc.vector.tensor_tensor(a1[hf][:], omz[hf][:], n_col[hf][:], op=OP.mult)
                nc.vector.tensor_tensor(a2[hf][:], z_col[hf][:], h_t[hf][:], op=OP.mult)
                nc.vector.tensor_tensor(h_t[hf][:], a1[hf][:], a2[hf][:], op=OP.add)

                q_ps = ppool1.tile([H, 128], F32, name=f"q_ps{hf}", tag=f"gq{hf}")
                nc.tensor.matmul(q_ps[:], W2T[:], h_t[hf][:], start=True, stop=True)
                nc.vector.tensor_copy(q_t[hf][:], q_ps[:])

            # prologue: advance half0 once (step 0) so the invariant holds
            gru_half(0)

            with tc.For_i(0, T_steps) as iv:
                dec_ps_t = [None]

                def tail_b2(b2):
                    nc.vector.tensor_tensor(m_sb[:, b2, :], att_sb[:, b2, :],
                                            maskadd[:, b2, :], op=OP.add)
                    nc.vector.max(mx8[b2][:], m_sb[:, b2, :])
                    nc.vector.max_index(ix8[b2][:], mx8[b2][:], m_sb[:, b2, :])
                    nc.vector.tensor_copy(idxf[b2][:], ix8[b2][:, 0:1])
                    nc.vector.tensor_copy(idxbuf[b2][:, ds(iv, 1)], idxf[b2][:])
                    nc.vector.tensor_scalar(negmax[b2][:], mx8[b2][:, 0:1], -1.0, None,
                                            op0=OP.mult)
                    nc.scalar.activation(m_sb[:, b2, :], m_sb[:, b2, :], AF.Exp,
                                         bias=negmax[b2][:, 0:1],
                                         accum_out=sexpbuf[b2][:, ds(iv, 1)])
                    oh = oh_t[b2]
                    nc.vector.tensor_scalar(oh[:], iota_f[:], idxf[b2][:, 0:1], None,
                                            op0=OP.is_equal)
                    for f in range(F):
                        nc.vector.tensor_tensor(gprod[b2][:, f, :], rawT[:, b2, f, :],
                                                oh[:], op=OP.mult)
                    nc.vector.reduce_sum(dec_b[b2][:], gprod[b2][:],
                                         axis=mybir.AxisListType.X)
                    nc.vector.tensor_scalar(oh[:], oh[:], NEG, None, op0=OP.mult)
                    nc.vector.tensor_tensor(maskadd[:, b2, :], maskadd[:, b2, :], oh[:],
                                            op=OP.add)
                    if dec_ps_t[0] is None:
                        dec_ps_t[0] = ppool1.tile([F, B], F32, name="dec_ps", tag="decps")
                    nc.tensor.transpose(dec_ps_t[0][:, b2 * 128:(b2 + 1) * 128],
                                        dec_b[b2][:], ident[:])
                    nc.vector.tensor_copy(dec_t[b2][:], dec_ps_t[0][:, b2 * 128:(b2 + 1) * 128])

                # phase A: advance half1 (concurrent with sweep half0)
                gru_half(1)

                # ---- attention sweep ----
                for ci, (st, sz) in enumerate(chunks):
                    nb = sz // N  # 8
                    b0 = st // N
                    hf = 0 if ci < 16 else 1
                    qv = q_t[hf][:, (b0 % 128):(b0 % 128) + nb]
                    if ci < NRES:
                        fpv = fp_res[:, ci * CHUNK:ci * CHUNK + sz]
                    else:
                        ring = fpool.tile([H, CHUNK], F32, name=f"ring{ci}", tag="fpring")
                        nc.sync.dma_start(ring[:], fp_dram[:, st:st + sz])
                        fpv = ring[:]
                    zc = zpool.tile([H, CHUNK], F32, name=f"zc{ci}", tag="zring")
                    eng = nc.vector if ci in dve_set else nc.gpsimd
                    eng.tensor_tensor(
                        zc[:].rearrange("h (b n) -> h b n", n=N),
                        fpv.rearrange("h (b n) -> h b n", n=N),
                        qv.broadcast_to([H, nb, N]),
                        op=OP.add)
                    nc.scalar.activation(zc[:], zc[:], AF.Tanh)
                    aps = ppool.tile([128, GS], F32, name=f"aps{ci}", tag="attps")
                    for j in range(4):
                        nc.tensor.matmul(
                            aps[32 * j:32 * (j + 1), :],
                            v32[:, :], zc[:, j * GS:(j + 1) * GS],
                            start=True, stop=True, tile_position=(0, 32 * j),
                            skip_group_check=True)
                    astg = apool.tile([128, GS], F32, name=f"astg{ci}", tag="astg")
                    nc.vector.tensor_copy(astg[:], aps[:])
                    nc.sync.dma_start(
                        att_sb[b0 % 128:b0 % 128 + nb, b0 // 128, :],
                        astg[0:128:32, :].rearrange("p (b n) -> p b n", n=N))
                    if ci == 15:
                        tail_b2(0)
                        # half0 GRU for the NEXT step overlaps sweep half1
                        gru_half(0)
                tail_b2(1)

            # ================= EPILOGUE =================
            for b2 in range(2):
                nc.scalar.activation(sexpbuf[b2][:], sexpbuf[b2][:], AF.Ln)
                nc.vector.tensor_scalar(sexpbuf[b2][:], sexpbuf[b2][:], -1.0, None,
                                        op0=OP.mult)
                nc.sync.dma_start(out_ext.ap()[0, b2 * 128:(b2 + 1) * 128, :], idxbuf[b2][:])
                nc.sync.dma_start(out_ext.ap()[1, b2 * 128:(b2 + 1) * 128, :], sexpbuf[b2][:])

    nc.finalize()
    return nc


def shard_inputs(inputs: dict) -> list[dict]:
    maps = []
    for c in range(8):
        m = {}
        for k, a in inputs.items():
            a = np.ascontiguousarray(np.asarray(a, dtype=np.float32))
            if k == "raw_features":
                m[k] = np.ascontiguousarray(a[c * B:(c + 1) * B])
            else:
                m[k] = a
        maps.append(m)
    return maps


def unshard_output(results: list[dict], T_steps=N - 1) -> np.ndarray:
    idx = np.concatenate([r["out"][0] for r in results], axis=0)
    logp = np.concatenate([r["out"][1] for r in results], axis=0)
    return np.stack([idx, logp])


_NC_CACHE = {}


def kernel(**inputs):
    """Full-batch entry: shard B=2048 across 8 NeuronCores, run, gather."""
    from concourse.bass_utils import run_bass_kernel_spmd
    if "nc" not in _NC_CACHE:
        _NC_CACHE["nc"] = build()
    nc = _NC_CACHE["nc"]
    in_maps = shard_inputs(inputs)
    res = run_bass_kernel_spmd(nc, in_maps, core_ids=list(range(8)))
    return unshard_output(res.results)

